# revision 21
# baseline (speedup 1.0000x reference)
"""Trainium2 Bass kernel for conv-projected multi-head attention.

Per batch item b of 8 (one NeuronCore each):
  y   = BN(depthwise3x3(x_b reshaped to [C,32,32]))      # q = k = v = y
  q/k/v = y @ w{q,k,v}^T  (12 heads x 32)
  att = softmax((q @ k^T) * sqrt(32));  out = (att @ v) @ wo^T

The softmax exp on the ACT engine is the critical resource (12.58M
elements/core at 1 elem/lane/cycle @1.2GHz ~ 96us), so the kernel keeps
ACT near-continuously busy and hides the other engines under it:
 - 6 blocks (2 l-halves x 3 head groups) of 8 t-tiles; per tile the
   scores matmuls (4 heads row-packed in the PE array, one PSUM bank
   each) feed one big exp activation [128, 2048].
 - PV accumulates over t with the values augmented by a ones column, so
   softmax denominators appear in psum rows 32/96 for free; the 4 heads
   of a group pack into 2 psum banks (even head partitions 0-33, odd
   64-97).
 - denominators: one approximate-reciprocal DVE op straight from PSUM,
   partition-broadcast by an indicator matmul, one multiply per bank
   into pair-packed attn tiles; wo is host-permuted/zero-padded to
   match that layout, so the output projection is plain matmuls.
 - a tunable subset of exp tiles runs on the DVE instead of ACT via a
   Schraudolph-style bitcast exp in bf16 (scores*scale stay in ~[-3.1,
   3.1]); the value path (E, V) is bf16 throughout (same PE rate).
"""
import sys

sys.path.insert(0, "/opt/trn_rl_repo")
from contextlib import ExitStack

import numpy as np

B, T, C = 8, 1024, 384
NH, DH = 12, 32
HH = WW = 32
SCALE = float(DH) ** 0.5
BN_EPS = 1e-5
NCORES = 8

CT = C // 128        # 3 c-tiles / head groups of 4
TT = T // 128        # 8 t-tiles
LCW = 512            # l-chunk width
LCN = T // LCW       # 2 l-chunks

# Schraudolph exp in bf16: exp(s*SCALE) ~ bitcast(int16(A*s + B)).
# Tiles with (per-block) tt index listed here run on DVE instead of ACT.
SCHRAUD_TT = (2, 5)
SCHRAUD_A = float(2.0 ** 7 / np.log(2.0) * SCALE)
SCHRAUD_B = float(127.0 * 2 ** 7 - 8.33)

_CACHE = {}


def _build(schraud_tt=SCHRAUD_TT, fast_recip=True, stage=5, debug=False):
    import concourse.bass as bass
    import concourse.tile as tile
    from concourse import bacc, mybir
    from concourse.masks import make_identity

    F32 = mybir.dt.float32
    F32R = mybir.dt.float32r
    I16 = mybir.dt.int16
    BF16 = mybir.dt.bfloat16
    AF = mybir.ActivationFunctionType
    ALU = mybir.AluOpType

    nc = bacc.Bacc("TRN2", target_bir_lowering=False, debug=False)

    xt_d = nc.dram_tensor("xt", [C, T], F32R, kind="ExternalInput").ap()
    w9_d = nc.dram_tensor("w9", [C, 9], F32, kind="ExternalInput").ap()
    bias_d = nc.dram_tensor("bias", [C, 1], F32, kind="ExternalInput").ap()
    wqT_d = nc.dram_tensor("wqT", [C, C], F32R, kind="ExternalInput").ap()
    wkT_d = nc.dram_tensor("wkT", [C, C], F32R, kind="ExternalInput").ap()
    wvT_d = nc.dram_tensor("wvT", [C, C], F32R, kind="ExternalInput").ap()
    woP_d = nc.dram_tensor("woP", [6 * 128, C], F32R, kind="ExternalInput").ap()
    bind_d = nc.dram_tensor("bind", [128, 128], F32R, kind="ExternalInput").ap()
    outT_d = nc.dram_tensor("outT", [C, T], F32, kind="ExternalOutput").ap()

    with tile.TileContext(nc) as tc, ExitStack() as top:
        persist = top.enter_context(tc.tile_pool(name="persist", bufs=1))

        # ---- persistent SBUF ----
        y_sb = [persist.tile([128, T], F32R, tag=f"y{i}", name=f"y{i}") for i in range(CT)]
        qT_sb = [persist.tile([128, T], F32R, tag=f"q{i}", name=f"q{i}") for i in range(CT)]
        kT_sb = [persist.tile([128, T], F32R, tag=f"k{i}", name=f"k{i}") for i in range(CT)]
        vaug = [persist.tile([128, NH, 34], BF16, tag=f"va{i}", name=f"va{i}") for i in range(TT)]
        # attention output, pair-packed: attnP[g][p] partitions 0-31 hold head
        # (4g+2p) d-rows, partitions 64-95 head (4g+2p+1); rest garbage that
        # the zero rows of woP annihilate.
        attnP = [
            [persist.tile([128, T], F32R, tag=f"at{g}{p}", name=f"at{g}{p}") for p in range(2)]
            for g in range(CT)
        ]
        wT_sb = {}
        for nm in ("q", "k", "v"):
            wT_sb[nm] = [
                persist.tile([128, C], F32R, tag=f"w{nm}{i}", name=f"w{nm}{i}")
                for i in range(CT)
            ]
        woP_sb = [persist.tile([128, C], F32R, tag=f"wo{j}", name=f"wo{j}") for j in range(6)]
        bind_sb = persist.tile([128, 128], F32R, tag="bind", name="bind")
        bias_sb = [persist.tile([128, 1], F32, tag=f"b{i}", name=f"b{i}") for i in range(CT)]

        # ---- phase 1: conv + projections ----
        with ExitStack() as ph1:
            convpool = ph1.enter_context(tc.tile_pool(name="convpool", bufs=1))
            conv_ps = ph1.enter_context(tc.tile_pool(name="conv_ps", bufs=2, space="PSUM"))
            qk_ps = ph1.enter_context(tc.tile_pool(name="qk_ps", bufs=2, space="PSUM"))
            v_ps = ph1.enter_context(tc.tile_pool(name="v_ps", bufs=2, space="PSUM"))

            xt_sb = [convpool.tile([128, T], F32R, tag=f"xt{i}", name=f"xt{i}") for i in range(CT)]
            xp = [convpool.tile([128, 34 * 34], F32R, tag=f"xp{i}", name=f"xp{i}") for i in range(CT)]
            w9_sb = [convpool.tile([128, 9], F32, tag=f"w9{i}", name=f"w9s{i}") for i in range(CT)]
            ident = convpool.tile([128, 128], F32, tag="ident")
            diag = [convpool.tile([128, 9, 128], F32R, tag=f"dg{i}", name=f"dg{i}") for i in range(CT)]

            make_identity(nc, ident[:])
            for i in range(CT):
                nc.sync.dma_start(xt_sb[i][:], xt_d[i * 128 : (i + 1) * 128, :])
                nc.sync.dma_start(w9_sb[i][:], w9_d[i * 128 : (i + 1) * 128, :])
                nc.vector.memset(xp[i][:].bitcast(F32), 0.0)
                nc.vector.tensor_copy(
                    xp[i][:].rearrange("p (a b) -> p a b", a=34)[:, 1:33, 1:33],
                    xt_sb[i][:].rearrange("p (a b) -> p a b", a=32),
                )
                for k in range(9):
                    nc.gpsimd.tensor_scalar_mul(
                        diag[i][:, k, :], ident[:], w9_sb[i][:, k : k + 1]
                    )
                nc.sync.dma_start(bias_sb[i][:], bias_d[i * 128 : (i + 1) * 128, :])

            # weight DMAs after conv inputs so conv starts early
            for nm, d in (("q", wqT_d), ("k", wkT_d), ("v", wvT_d)):
                for i in range(CT):
                    nc.sync.dma_start(wT_sb[nm][i][:], d[i * 128 : (i + 1) * 128, :])
            for j in range(6):
                nc.sync.dma_start(woP_sb[j][:], woP_d[j * 128 : (j + 1) * 128, :])
            nc.sync.dma_start(bind_sb[:], bind_d)

            # conv: 9 accumulating diag matmuls per (t-half, c-tile)
            for th in range(2):
                for i in range(CT):
                    yp = conv_ps.tile([128, 512], F32, tag="conv")
                    r0 = th * 16
                    for k in range(9):
                        dy, dx = k // 3 - 1, k % 3 - 1
                        off = (r0 + 1 + dy) * 34 + (1 + dx)
                        rhs = bass.AP(
                            tensor=xp[i].tensor,
                            offset=xp[i].offset + off,
                            ap=[list(p) for p in xp[i].ap[:1]] + [[34, 16], [1, 32]],
                        )
                        nc.tensor.matmul(
                            yp[:].rearrange("p (a b) -> p a b", a=16),
                            diag[i][:, k, :],
                            rhs,
                            start=(k == 0),
                            stop=(k == 8),
                        )
                    nc.vector.tensor_scalar_add(
                        y_sb[i][:, th * 512 : (th + 1) * 512], yp[:], bias_sb[i][:]
                    )

            # q/k projections: qT[o, t]
            for ot in range(CT):
                for nm, dst in (("q", qT_sb), ("k", kT_sb)):
                    for th in range(2):
                        pp = qk_ps.tile([128, 512], F32, tag="qk")
                        for kt in range(CT):
                            nc.tensor.matmul(
                                pp[:],
                                wT_sb[nm][kt][:, ot * 128 : (ot + 1) * 128],
                                y_sb[kt][:, th * 512 : (th + 1) * 512],
                                start=(kt == 0),
                                stop=(kt == CT - 1),
                            )
                        nc.vector.tensor_copy(dst[ot][:, th * 512 : (th + 1) * 512], pp[:])

            # v projection into vaug [t, h, 34] (cols 32/33 = ones, so PV
            # yields row sums and a finite pad row)
            for tt in range(TT):
                vp = v_ps.tile([128, C], F32, tag="v")
                for kt in range(CT):
                    nc.tensor.matmul(
                        vp[:],
                        y_sb[kt][:, tt * 128 : (tt + 1) * 128],
                        wT_sb["v"][kt][:],
                        start=(kt == 0),
                        stop=(kt == CT - 1),
                    )
                nc.gpsimd.memset(vaug[tt][:, :, 32:34], 1.0)
                nc.vector.tensor_copy(
                    vaug[tt][:, :, 0:32], vp[:].rearrange("p (h d) -> p h d", h=NH)
                )

        # ---- phase 2: attention ----
        with ExitStack() as ph2:
            s_ps = ph2.enter_context(tc.tile_pool(name="s_ps", bufs=1, space="PSUM"))
            ov_ps = ph2.enter_context(tc.tile_pool(name="ov_ps", bufs=1, space="PSUM"))
            o_ps = ph2.enter_context(tc.tile_pool(name="o_ps", bufs=1, space="PSUM"))
            rb_ps = ph2.enter_context(tc.tile_pool(name="rb_ps", bufs=1, space="PSUM"))
            epool = ph2.enter_context(tc.tile_pool(name="epool", bufs=3))
            rrpool = ph2.enter_context(tc.tile_pool(name="rrpool", bufs=2))
            rbpool = ph2.enter_context(tc.tile_pool(name="rbpool", bufs=2))
            ocpool = ph2.enter_context(tc.tile_pool(name="ocpool", bufs=2))

            # one persistent 2-bank ov: bank p holds pair p (head 4g+2p at
            # partitions 0-33, head 4g+2p+1 at 64-97).  The rows no PV matmul
            # ever writes are set to 1.0 once so the full-tile reciprocal
            # stays finite (their bind weights are zero).
            ov = ov_ps.tile([128, 2 * LCW], F32, tag="ov", name="ov")
            nc.vector.memset(ov[32:64, :], 1.0)
            nc.vector.memset(ov[96:128, :], 1.0)

            def out_proj(lc):
                for ot in range(CT):
                    op = o_ps.tile([128, LCW], F32, tag="op")
                    for j in range(6):
                        g, p = j // 2, j % 2
                        nc.tensor.matmul(
                            op[:],
                            woP_sb[j][:, ot * 128 : (ot + 1) * 128],
                            attnP[g][p][:, lc * LCW : (lc + 1) * LCW],
                            start=(j == 0),
                            stop=(j == 5),
                        )
                    oc = ocpool.tile([128, LCW], F32, tag="oc")
                    nc.vector.tensor_copy(oc[:], op[:])
                    nc.sync.dma_start(
                        outT_d[ot * 128 : (ot + 1) * 128, lc * LCW : (lc + 1) * LCW],
                        oc[:],
                    )

            for lc in range(LCN) if stage >= 2 else []:
                for g in range(CT):
                    for tt in range(TT):
                        s4 = s_ps.tile([128, 2048], F32, tag="s4")
                        for hl in range(4):
                            nc.tensor.matmul(
                                s4[:, 512 * hl : 512 * (hl + 1)],
                                kT_sb[g][32 * hl : 32 * (hl + 1), tt * 128 : (tt + 1) * 128],
                                qT_sb[g][32 * hl : 32 * (hl + 1), lc * LCW : (lc + 1) * LCW],
                                start=True,
                                stop=True,
                                tile_position=(32 * hl, 0),
                            )
                        e = epool.tile([128, 2048], BF16, tag="E")
                        if tt in schraud_tt:
                            nc.vector.tensor_scalar(
                                e[:].bitcast(I16),
                                s4[:],
                                SCHRAUD_A,
                                SCHRAUD_B,
                                ALU.mult,
                                ALU.add,
                            )
                        else:
                            nc.scalar.activation(e[:], s4[:], AF.Exp, scale=SCALE)
                        if stage < 3:
                            continue
                        for hl in range(4):
                            pb = 64 * (hl % 2)
                            cb = LCW * (hl // 2)
                            nc.tensor.matmul(
                                ov[pb : pb + 34, cb : cb + LCW],
                                vaug[tt][:, 4 * g + hl, :],
                                e[:, 512 * hl : 512 * (hl + 1)],
                                start=(tt == 0),
                                stop=(tt == TT - 1),
                                tile_position=(0, pb),
                                # the sim's zero-region group bookkeeping
                                # mis-addresses partition-offset outputs;
                                # per-element pending-zero still applies
                                skip_group_check=True,
                            )
                    if stage < 4:
                        continue
                    # normalization: approximate reciprocal of the whole ov
                    # (only rows 32/96 are used), partition-broadcast via the
                    # bind indicator matmul, then one multiply per bank into
                    # the pair-packed attn tiles.
                    rr = rrpool.tile([128, 2 * LCW], F32, tag="rr")
                    if fast_recip:
                        nc.vector.reciprocal_approx_fast(rr[:], ov[:])
                    else:
                        nc.vector.reciprocal(rr[:], ov[:])
                    rrc = rrpool.tile([128, 2 * LCW], F32R, tag="rrc")
                    nc.vector.tensor_copy(rrc[:], rr[:])
                    for p in range(2):
                        rbp = rb_ps.tile([128, LCW], F32, tag="rbp")
                        nc.tensor.matmul(
                            rbp[:],
                            bind_sb[:],
                            rrc[:, LCW * p : LCW * (p + 1)],
                            start=True,
                            stop=True,
                        )
                        # DVE may read only one PSUM operand; ov is the other
                        rbs = rbpool.tile([128, LCW], F32, tag="rb")
                        nc.vector.tensor_copy(rbs[:], rbp[:])
                        nc.vector.tensor_tensor(
                            attnP[g][p][:, lc * LCW : (lc + 1) * LCW],
                            ov[:, LCW * p : LCW * (p + 1)],
                            rbs[:],
                            ALU.mult,
                        )
                    # stream the previous l-chunk's output projection
                    if g == 1 and lc >= 1 and stage >= 5:
                        out_proj(lc - 1)
            if stage >= 5:
                out_proj(LCN - 1)

    nc.compile()
    return nc


def _prep_inputs(x, conv_w, bn_gamma, bn_beta, bn_mean, bn_var, wq, wk, wv, wo):
    f32 = np.float32
    inv = (bn_gamma / np.sqrt(bn_var + BN_EPS)).astype(f32)
    w9 = (conv_w.reshape(C, 9) * inv[:, None]).astype(f32)
    bias = (bn_beta - bn_mean * inv).astype(f32).reshape(C, 1)
    wqT = np.ascontiguousarray(np.asarray(wq, f32).T)
    wkT = np.ascontiguousarray(np.asarray(wk, f32).T)
    wvT = np.ascontiguousarray(np.asarray(wv, f32).T)
    wo = np.asarray(wo, f32)
    # pair-packed out-projection weights: row r of woP[2g+p] multiplies
    # partition r of attnP[g][p]; heads 4g+2p (rows 0-31) and 4g+2p+1
    # (rows 64-95), zeros elsewhere.
    woP = np.zeros((6 * 128, C), f32)
    for g in range(CT):
        for p in range(2):
            j = 2 * g + p
            h0 = 4 * g + 2 * p
            woP[j * 128 + 0 : j * 128 + 32, :] = wo[:, 32 * h0 : 32 * h0 + 32].T
            woP[j * 128 + 64 : j * 128 + 96, :] = wo[:, 32 * h0 + 32 : 32 * h0 + 64].T
    # bind[k, p_out] = 1 iff k == 32 + 64*(p_out >= 64): broadcasts the two
    # sums rows of an ov bank across their 64-partition halves.
    bind = np.zeros((128, 128), f32)
    bind[32, 0:64] = 1.0
    bind[96, 64:128] = 1.0
    maps = []
    for b in range(B):
        maps.append(
            {
                "xt": np.ascontiguousarray(np.asarray(x[b], f32).T),
                "w9": w9,
                "bias": bias,
                "wqT": wqT,
                "wkT": wkT,
                "wvT": wvT,
                "woP": woP,
                "bind": bind,
            }
        )
    return maps


def kernel(x, conv_w, bn_gamma, bn_beta, bn_mean, bn_var, wq, wk, wv, wo, h, w,
           **kw):
    assert int(h) == HH and int(w) == WW
    from concourse.bass_utils import run_bass_kernel_spmd

    if "nc" not in _CACHE:
        _CACHE["nc"] = _build()
    nc = _CACHE["nc"]
    maps = _prep_inputs(
        x, conv_w, bn_gamma, bn_beta, bn_mean, bn_var, wq, wk, wv, wo
    )
    res = run_bass_kernel_spmd(nc, maps, list(range(NCORES)))
    out = np.stack([res.results[b]["outT"].T for b in range(B)])
    return out.astype(np.float32)


# revision 23
# speedup vs baseline: 1.0783x; 1.0783x over previous
"""Trainium2 Bass kernel for conv-projected multi-head attention.

Per batch item b of 8 (one NeuronCore each):
  y   = BN(depthwise3x3(x_b reshaped to [C,32,32]))      # q = k = v = y
  q/k/v = y @ w{q,k,v}^T  (12 heads x 32)
  att = softmax((q @ k^T) * sqrt(32));  out = (att @ v) @ wo^T

The softmax exp on the ACT engine is the critical resource (12.58M
elements/core at 1 elem/lane/cycle @1.2GHz ~ 96us), so the kernel keeps
ACT near-continuously busy and hides the other engines under it:
 - 6 blocks (2 l-halves x 3 head groups) of 8 t-tiles; per tile the
   scores matmuls (4 heads row-packed in the PE array, one PSUM bank
   each) feed one big exp activation [128, 2048].
 - PV accumulates over t with the values augmented by a ones column, so
   softmax denominators appear in psum rows 32/96 for free; the 4 heads
   of a group pack into 2 psum banks (even head partitions 0-33, odd
   64-97).
 - denominators: one approximate-reciprocal DVE op straight from PSUM,
   partition-broadcast by an indicator matmul, one multiply per bank
   into pair-packed attn tiles; wo is host-permuted/zero-padded to
   match that layout, so the output projection is plain matmuls.
 - a tunable subset of exp tiles runs on the DVE instead of ACT via a
   Schraudolph-style bitcast exp in bf16 (scores*scale stay in ~[-3.1,
   3.1]); the value path (E, V) is bf16 throughout (same PE rate).
"""
import sys

sys.path.insert(0, "/opt/trn_rl_repo")
from contextlib import ExitStack

import numpy as np

B, T, C = 8, 1024, 384
NH, DH = 12, 32
HH = WW = 32
SCALE = float(DH) ** 0.5
BN_EPS = 1e-5
NCORES = 8

CT = C // 128        # 3 c-tiles / head groups of 4
TT = T // 128        # 8 t-tiles
LCW = 512            # l-chunk width
LCN = T // LCW       # 2 l-chunks

# Schraudolph exp in bf16: exp(s*SCALE) ~ bitcast(int16(A*s + B)).
# Pair-tiles with (per-block) flat index 2*tt+pair listed here run on the
# DVE instead of ACT, balancing the two engines.
SCHRAUD_TT = (5, 10, 15)
SCHRAUD_A = float(2.0 ** 7 / np.log(2.0) * SCALE)
SCHRAUD_B = float(127.0 * 2 ** 7 - 8.33)

_CACHE = {}


def _build(schraud_tt=SCHRAUD_TT, fast_recip=True, stage=5, debug=False):
    import concourse.bass as bass
    import concourse.tile as tile
    from concourse import bacc, mybir
    from concourse.masks import make_identity

    F32 = mybir.dt.float32
    F32R = mybir.dt.float32r
    I16 = mybir.dt.int16
    BF16 = mybir.dt.bfloat16
    AF = mybir.ActivationFunctionType
    ALU = mybir.AluOpType

    nc = bacc.Bacc("TRN2", target_bir_lowering=False, debug=False)

    xt_d = nc.dram_tensor("xt", [C, T], F32R, kind="ExternalInput").ap()
    w9_d = nc.dram_tensor("w9", [C, 9], F32, kind="ExternalInput").ap()
    bias_d = nc.dram_tensor("bias", [C, 1], F32, kind="ExternalInput").ap()
    wqT_d = nc.dram_tensor("wqT", [C, C], F32R, kind="ExternalInput").ap()
    wkT_d = nc.dram_tensor("wkT", [C, C], F32R, kind="ExternalInput").ap()
    wvT_d = nc.dram_tensor("wvT", [C, C], F32R, kind="ExternalInput").ap()
    woP_d = nc.dram_tensor("woP", [6 * 128, C], F32R, kind="ExternalInput").ap()
    bind_d = nc.dram_tensor("bind", [128, 128], F32R, kind="ExternalInput").ap()
    outT_d = nc.dram_tensor("outT", [C, T], F32, kind="ExternalOutput").ap()

    with tile.TileContext(nc) as tc, ExitStack() as top:
        persist = top.enter_context(tc.tile_pool(name="persist", bufs=1))

        # ---- persistent SBUF ----
        y_sb = [persist.tile([128, T], F32R, tag=f"y{i}", name=f"y{i}") for i in range(CT)]
        qT_sb = [persist.tile([128, T], F32R, tag=f"q{i}", name=f"q{i}") for i in range(CT)]
        kT_sb = [persist.tile([128, T], F32R, tag=f"k{i}", name=f"k{i}") for i in range(CT)]
        vaug = [persist.tile([128, NH, 34], BF16, tag=f"va{i}", name=f"va{i}") for i in range(TT)]
        # attention output, pair-packed: attnP[g][p] partitions 0-31 hold head
        # (4g+2p) d-rows, partitions 64-95 head (4g+2p+1); rest garbage that
        # the zero rows of woP annihilate.
        attnP = [
            [persist.tile([128, T], F32R, tag=f"at{g}{p}", name=f"at{g}{p}") for p in range(2)]
            for g in range(CT)
        ]
        wT_sb = {}
        for nm in ("q", "k", "v"):
            wT_sb[nm] = [
                persist.tile([128, C], F32R, tag=f"w{nm}{i}", name=f"w{nm}{i}")
                for i in range(CT)
            ]
        woP_sb = [persist.tile([128, C], F32R, tag=f"wo{j}", name=f"wo{j}") for j in range(6)]
        bind_sb = persist.tile([128, 128], F32R, tag="bind", name="bind")
        bias_sb = [persist.tile([128, 1], F32, tag=f"b{i}", name=f"b{i}") for i in range(CT)]

        # ---- phase 1: conv + projections ----
        with ExitStack() as ph1:
            convpool = ph1.enter_context(tc.tile_pool(name="convpool", bufs=1))
            conv_ps = ph1.enter_context(tc.tile_pool(name="conv_ps", bufs=2, space="PSUM"))
            qk_ps = ph1.enter_context(tc.tile_pool(name="qk_ps", bufs=2, space="PSUM"))
            v_ps = ph1.enter_context(tc.tile_pool(name="v_ps", bufs=2, space="PSUM"))

            xt_sb = [convpool.tile([128, T], F32R, tag=f"xt{i}", name=f"xt{i}") for i in range(CT)]
            xp = [convpool.tile([128, 34 * 34], F32R, tag=f"xp{i}", name=f"xp{i}") for i in range(CT)]
            w9_sb = [convpool.tile([128, 9], F32, tag=f"w9{i}", name=f"w9s{i}") for i in range(CT)]
            ident = convpool.tile([128, 128], F32, tag="ident")
            diag = [convpool.tile([128, 9, 128], F32R, tag=f"dg{i}", name=f"dg{i}") for i in range(CT)]

            make_identity(nc, ident[:])
            for i in range(CT):
                nc.sync.dma_start(xt_sb[i][:], xt_d[i * 128 : (i + 1) * 128, :])
                nc.sync.dma_start(w9_sb[i][:], w9_d[i * 128 : (i + 1) * 128, :])
                nc.vector.memset(xp[i][:].bitcast(F32), 0.0)
                nc.vector.tensor_copy(
                    xp[i][:].rearrange("p (a b) -> p a b", a=34)[:, 1:33, 1:33],
                    xt_sb[i][:].rearrange("p (a b) -> p a b", a=32),
                )
                for k in range(9):
                    nc.vector.tensor_scalar_mul(
                        diag[i][:, k, :], ident[:], w9_sb[i][:, k : k + 1]
                    )
                nc.sync.dma_start(bias_sb[i][:], bias_d[i * 128 : (i + 1) * 128, :])

            # weight DMAs after conv inputs so conv starts early
            for nm, d in (("q", wqT_d), ("k", wkT_d), ("v", wvT_d)):
                for i in range(CT):
                    nc.sync.dma_start(wT_sb[nm][i][:], d[i * 128 : (i + 1) * 128, :])
            for j in range(6):
                nc.sync.dma_start(woP_sb[j][:], woP_d[j * 128 : (j + 1) * 128, :])
            nc.sync.dma_start(bind_sb[:], bind_d)

            # conv: 9 accumulating diag matmuls per (t-half, c-tile)
            for th in range(2):
                for i in range(CT):
                    yp = conv_ps.tile([128, 512], F32, tag="conv")
                    r0 = th * 16
                    for k in range(9):
                        dy, dx = k // 3 - 1, k % 3 - 1
                        off = (r0 + 1 + dy) * 34 + (1 + dx)
                        rhs = bass.AP(
                            tensor=xp[i].tensor,
                            offset=xp[i].offset + off,
                            ap=[list(p) for p in xp[i].ap[:1]] + [[34, 16], [1, 32]],
                        )
                        nc.tensor.matmul(
                            yp[:].rearrange("p (a b) -> p a b", a=16),
                            diag[i][:, k, :],
                            rhs,
                            start=(k == 0),
                            stop=(k == 8),
                        )
                    nc.vector.tensor_scalar_add(
                        y_sb[i][:, th * 512 : (th + 1) * 512], yp[:], bias_sb[i][:]
                    )

            # q/k projections for head group 0 (groups 1/2 are emitted as
            # PE filler inside the first attention blocks)
            for ot in range(1):
                for nm, dst in (("q", qT_sb), ("k", kT_sb)):
                    for th in range(2):
                        pp = qk_ps.tile([128, 512], F32, tag="qk")
                        for kt in range(CT):
                            nc.tensor.matmul(
                                pp[:],
                                wT_sb[nm][kt][:, ot * 128 : (ot + 1) * 128],
                                y_sb[kt][:, th * 512 : (th + 1) * 512],
                                start=(kt == 0),
                                stop=(kt == CT - 1),
                            )
                        nc.vector.tensor_copy(dst[ot][:, th * 512 : (th + 1) * 512], pp[:])

            # v projection into vaug [t, h, 34] (cols 32/33 = ones, so PV
            # yields row sums and a finite pad row)
            for tt in range(TT):
                vp = v_ps.tile([128, C], F32, tag="v")
                for kt in range(CT):
                    nc.tensor.matmul(
                        vp[:],
                        y_sb[kt][:, tt * 128 : (tt + 1) * 128],
                        wT_sb["v"][kt][:],
                        start=(kt == 0),
                        stop=(kt == CT - 1),
                    )
                nc.vector.memset(vaug[tt][:, :, 32:34], 1.0)
                nc.vector.tensor_copy(
                    vaug[tt][:, :, 0:32], vp[:].rearrange("p (h d) -> p h d", h=NH)
                )

        # ---- phase 2: attention ----
        with ExitStack() as ph2:
            s_ps = ph2.enter_context(tc.tile_pool(name="s_ps", bufs=1, space="PSUM"))
            ov_ps = ph2.enter_context(tc.tile_pool(name="ov_ps", bufs=1, space="PSUM"))
            o_ps = ph2.enter_context(tc.tile_pool(name="o_ps", bufs=1, space="PSUM"))
            rb_ps = ph2.enter_context(tc.tile_pool(name="rb_ps", bufs=1, space="PSUM"))
            epool = ph2.enter_context(tc.tile_pool(name="epool", bufs=3))
            rrpool = ph2.enter_context(tc.tile_pool(name="rrpool", bufs=2))
            ovspool = ph2.enter_context(tc.tile_pool(name="ovspool", bufs=2))
            ocpool = ph2.enter_context(tc.tile_pool(name="ocpool", bufs=2))

            # one persistent 2-bank ov: bank p holds pair p (head 4g+2p at
            # partitions 0-33, head 4g+2p+1 at 64-97).  The rows no PV matmul
            # ever writes are set to 1.0 once so the full-tile reciprocal
            # stays finite (their bind weights are zero).
            ov = ov_ps.tile([128, 2 * LCW], F32, tag="ov", name="ov")
            nc.vector.memset(ov[32:64, :], 1.0)
            nc.vector.memset(ov[96:128, :], 1.0)

            def out_proj(lc):
                for ot in range(CT):
                    op = o_ps.tile([128, LCW], F32, tag="op")
                    for j in range(6):
                        g, p = j // 2, j % 2
                        nc.tensor.matmul(
                            op[:],
                            woP_sb[j][:, ot * 128 : (ot + 1) * 128],
                            attnP[g][p][:, lc * LCW : (lc + 1) * LCW],
                            start=(j == 0),
                            stop=(j == 5),
                        )
                    oc = ocpool.tile([128, LCW], F32, tag="oc")
                    nc.vector.tensor_copy(oc[:], op[:])
                    nc.sync.dma_start(
                        outT_d[ot * 128 : (ot + 1) * 128, lc * LCW : (lc + 1) * LCW],
                        oc[:],
                    )

            # leftover projection work, emitted in small chunks as PE
            # filler between attention tiles (keeps the PE warm while ACT
            # runs exp and shortens the pre-attention serial stretch)
            filler = []
            for ot in (1, 2):
                for nm, dst in (("q", qT_sb), ("k", kT_sb)):
                    for th in range(2):
                        def _chunk(ot=ot, nm=nm, dst=dst, th=th):
                            pp = o_ps.tile([128, 512], F32, tag="op")
                            for kt in range(CT):
                                nc.tensor.matmul(
                                    pp[:],
                                    wT_sb[nm][kt][:, ot * 128 : (ot + 1) * 128],
                                    y_sb[kt][:, th * 512 : (th + 1) * 512],
                                    start=(kt == 0),
                                    stop=(kt == CT - 1),
                                )
                            nc.vector.tensor_copy(
                                dst[ot][:, th * 512 : (th + 1) * 512], pp[:]
                            )
                        filler.append(_chunk)
            norm_pending = []

            for lc in range(LCN) if stage >= 2 else []:
                for g in range(CT):
                    for tt in range(TT):
                        # two pair-tiles per t-tile: each head keeps its own
                        # psum bank, and bufs=2 lets the next tile's scores
                        # run under this tile's exp (gapless ACT)
                        es = []
                        for pr in range(2):
                            s4 = s_ps.tile([128, 1024], F32, tag="s4")
                            for hh in range(2):
                                hl = 2 * pr + hh
                                nc.tensor.matmul(
                                    s4[:, 512 * hh : 512 * (hh + 1)],
                                    kT_sb[g][32 * hl : 32 * (hl + 1), tt * 128 : (tt + 1) * 128],
                                    qT_sb[g][32 * hl : 32 * (hl + 1), lc * LCW : (lc + 1) * LCW],
                                    start=True,
                                    stop=True,
                                    tile_position=(32 * hl, 0),
                                )
                            e = epool.tile([128, 1024], BF16, tag="E")
                            if 2 * tt + pr in schraud_tt:
                                nc.vector.tensor_scalar(
                                    e[:].bitcast(I16),
                                    s4[:],
                                    SCHRAUD_A,
                                    SCHRAUD_B,
                                    ALU.mult,
                                    ALU.add,
                                )
                            else:
                                nc.scalar.activation(e[:], s4[:], AF.Exp, scale=SCALE)
                            es.append(e)
                        if norm_pending and tt == 2:
                            norm_pending.pop(0)()
                        if stage < 3:
                            continue
                        for hl in range(4):
                            pb = 64 * (hl % 2)
                            cb = LCW * (hl // 2)
                            nc.tensor.matmul(
                                ov[pb : pb + 34, cb : cb + LCW],
                                vaug[tt][:, 4 * g + hl, :],
                                es[hl // 2][:, 512 * (hl % 2) : 512 * (hl % 2 + 1)],
                                start=(tt == 0),
                                stop=(tt == TT - 1),
                                tile_position=(0, pb),
                                # the sim's zero-region group bookkeeping
                                # mis-addresses partition-offset outputs;
                                # per-element pending-zero still applies
                                skip_group_check=True,
                            )
                        if filler and tt % 2 == 1:
                            filler.pop(0)()
                    if stage < 4:
                        continue
                    # eager normalization part: reciprocal of the sums rows
                    # and an ov->SBUF copy, so the next block's PV can claim
                    # the ov banks after just two DVE ops
                    rr = rrpool.tile([128, 2 * LCW], F32, tag="rr")
                    if fast_recip:
                        nc.vector.reciprocal_approx_fast(rr[:], ov[:])
                    else:
                        nc.vector.reciprocal(rr[:], ov[:])
                    ovs = ovspool.tile([128, 2 * LCW], F32, tag="ovs")
                    nc.vector.tensor_copy(ovs[:], ov[:])

                    # lazy part (emitted early in the next block, so the bind
                    # matmuls never stall the PE): broadcast the reciprocals
                    # across partitions and scale the copied O values into the
                    # pair-packed attn tiles; the finished l-chunk then flows
                    # into the output projection.
                    def _lazy(lc=lc, g=g, rr=rr, ovs=ovs):
                        rrc = rrpool.tile([128, 2 * LCW], F32R, tag="rrc")
                        nc.vector.tensor_copy(rrc[:], rr[:])
                        for p in range(2):
                            rbp = rb_ps.tile([128, LCW], F32, tag="rbp")
                            nc.tensor.matmul(
                                rbp[:],
                                bind_sb[:],
                                rrc[:, LCW * p : LCW * (p + 1)],
                                start=True,
                                stop=True,
                            )
                            nc.vector.tensor_tensor(
                                attnP[g][p][:, lc * LCW : (lc + 1) * LCW],
                                ovs[:, LCW * p : LCW * (p + 1)],
                                rbp[:],
                                ALU.mult,
                            )
                        if g == CT - 1 and stage >= 5:
                            out_proj(lc)

                    norm_pending.append(_lazy)
            while norm_pending:
                norm_pending.pop(0)()

    nc.compile()
    return nc


def _prep_inputs(x, conv_w, bn_gamma, bn_beta, bn_mean, bn_var, wq, wk, wv, wo):
    f32 = np.float32
    inv = (bn_gamma / np.sqrt(bn_var + BN_EPS)).astype(f32)
    w9 = (conv_w.reshape(C, 9) * inv[:, None]).astype(f32)
    bias = (bn_beta - bn_mean * inv).astype(f32).reshape(C, 1)
    wqT = np.ascontiguousarray(np.asarray(wq, f32).T)
    wkT = np.ascontiguousarray(np.asarray(wk, f32).T)
    wvT = np.ascontiguousarray(np.asarray(wv, f32).T)
    wo = np.asarray(wo, f32)
    # pair-packed out-projection weights: row r of woP[2g+p] multiplies
    # partition r of attnP[g][p]; heads 4g+2p (rows 0-31) and 4g+2p+1
    # (rows 64-95), zeros elsewhere.
    woP = np.zeros((6 * 128, C), f32)
    for g in range(CT):
        for p in range(2):
            j = 2 * g + p
            h0 = 4 * g + 2 * p
            woP[j * 128 + 0 : j * 128 + 32, :] = wo[:, 32 * h0 : 32 * h0 + 32].T
            woP[j * 128 + 64 : j * 128 + 96, :] = wo[:, 32 * h0 + 32 : 32 * h0 + 64].T
    # bind[k, p_out] = 1 iff k == 32 + 64*(p_out >= 64): broadcasts the two
    # sums rows of an ov bank across their 64-partition halves.
    bind = np.zeros((128, 128), f32)
    bind[32, 0:64] = 1.0
    bind[96, 64:128] = 1.0
    maps = []
    for b in range(B):
        maps.append(
            {
                "xt": np.ascontiguousarray(np.asarray(x[b], f32).T),
                "w9": w9,
                "bias": bias,
                "wqT": wqT,
                "wkT": wkT,
                "wvT": wvT,
                "woP": woP,
                "bind": bind,
            }
        )
    return maps


def kernel(x, conv_w, bn_gamma, bn_beta, bn_mean, bn_var, wq, wk, wv, wo, h, w,
           **kw):
    assert int(h) == HH and int(w) == WW
    from concourse.bass_utils import run_bass_kernel_spmd

    if "nc" not in _CACHE:
        _CACHE["nc"] = _build()
    nc = _CACHE["nc"]
    maps = _prep_inputs(
        x, conv_w, bn_gamma, bn_beta, bn_mean, bn_var, wq, wk, wv, wo
    )
    res = run_bass_kernel_spmd(nc, maps, list(range(NCORES)))
    out = np.stack([res.results[b]["outT"].T for b in range(B)])
    return out.astype(np.float32)
